# revision 6
# baseline (speedup 1.0000x reference)
"""Trainium2 Bass kernel for the ConductanceLIFNetwork problem.

Strategy: column-shard the 1536 postsynaptic neurons across 8 cores (192
each), batch (32) replicated.  Per timestep each core computes its slice of
the recurrent conductance inputs with the spike vector as the PE stationary
operand (12 accumulating matmuls streaming 384 weight columns), runs the
fused LIF state updates on DVE/Pool, transposes its new spike slice on the
PE, and exchanges slices with the other cores through an AllGather so every
core has the full presynaptic spike vector for the next step.  The
feedforward input matmuls depend only on the (known) input spikes, so they
are issued first each step and execute while the AllGather is in flight.
"""

import math

import numpy as np

# ---- problem constants (hardcoded; kernel.py must be self-contained) ----
N_NEURONS = 1536
N_INPUTS = 768
BATCH = 32
T_STEPS = 256
N_CORES = 8
COLS = N_NEURONS // N_CORES  # 192 postsynaptic neurons per core
DT = 1.0

CELL_TAU_MEM = np.array([20.0, 10.0], np.float32)
CELL_TAUREF = np.array([2.0, 1.0], np.float32)
# theta=-50, u_reset=e_l=-65, g_l=10 for both cell types
SYN_TAU_RISE = np.array([0.5, 2.0, 0.5], np.float32)
SYN_TAU_DECAY = np.array([2.0, 100.0, 5.0], np.float32)

AR = [float(math.exp(-DT / t)) for t in SYN_TAU_RISE]   # x rise decays
AD = [float(math.exp(-DT / t)) for t in SYN_TAU_DECAY]  # g decay
ARF = float(math.exp(-DT / 0.5))
ADF = float(math.exp(-DT / 2.0))

K_REC = N_NEURONS // 128   # 12 contraction tiles for recurrent matmul
K_FF = N_INPUTS // 128     # 6 contraction tiles for feedforward matmul


def _build(T: int):
    import concourse.bacc as bacc
    import concourse.tile as tile
    import concourse.mybir as mybir

    f32 = mybir.dt.float32
    op = mybir.AluOpType

    nc = bacc.Bacc(
        "TRN2",
        target_bir_lowering=False,
        debug=False,
        enable_asserts=False,
        num_devices=N_CORES,
    )

    # ---- kernel I/O ----
    w_in = nc.dram_tensor("w_in", [K_REC, 128, 2 * COLS], f32, kind="ExternalInput").ap()
    wf_in = nc.dram_tensor("wf_in", [K_FF, 128, COLS], f32, kind="ExternalInput").ap()
    itT_in = nc.dram_tensor("itT_in", [K_FF, 128, T, BATCH], f32, kind="ExternalInput").ap()
    lc_in = nc.dram_tensor("lc_in", [BATCH, COLS], f32, kind="ExternalInput").ap()
    rs_in = nc.dram_tensor("rs_in", [BATCH, COLS], f32, kind="ExternalInput").ap()
    id_in = nc.dram_tensor("id_in", [BATCH, BATCH], f32, kind="ExternalInput").ap()
    out_s = nc.dram_tensor("out_s", [T, BATCH, COLS], f32, kind="ExternalOutput").ap()
    out_u = nc.dram_tensor("out_u", [T, BATCH, COLS], f32, kind="ExternalOutput").ap()

    with tile.TileContext(nc) as tc:
        with (
            tc.tile_pool(name="const", bufs=1) as cpool,
            tc.tile_pool(name="state", bufs=1) as spool,
            tc.tile_pool(name="st", bufs=2) as st_pool,
            tc.tile_pool(name="itt", bufs=4) as it_pool,
            tc.tile_pool(name="pin", bufs=2, space="PSUM") as pin_pool,
            tc.tile_pool(name="pff", bufs=2, space="PSUM") as pff_pool,
            tc.tile_pool(name="ptr", bufs=2, space="PSUM") as ptr_pool,
            tc.tile_pool(name="agi", bufs=2, space="DRAM") as agi_pool,
            tc.tile_pool(name="ago", bufs=2, space="DRAM") as ago_pool,
        ):
            # ---- load constants ----
            w_sb = cpool.tile([128, K_REC, 2 * COLS], f32)
            nc.sync.dma_start(w_sb[:], w_in.rearrange("k p c -> p k c"))
            wf_sb = cpool.tile([128, K_FF, COLS], f32)
            nc.sync.dma_start(wf_sb[:], wf_in.rearrange("k p c -> p k c"))
            lc_t = cpool.tile([BATCH, COLS], f32)
            nc.sync.dma_start(lc_t[:], lc_in)
            rs_t = cpool.tile([BATCH, COLS], f32)
            nc.sync.dma_start(rs_t[:], rs_in)
            ident = cpool.tile([BATCH, BATCH], f32)
            nc.sync.dma_start(ident[:], id_in)
            neg65 = cpool.tile([BATCH, COLS], f32)
            nc.vector.memset(neg65[:], -65.0)

            # ---- persistent state tiles ----
            def state(val=0.0):
                t_ = spool.tile([BATCH, COLS], f32, tag=f"st{state.i}")
                state.i += 1
                nc.vector.memset(t_[:], val)
                return t_
            state.i = 0

            U = state(-65.0)
            ref = state()
            x0, x1, x2 = state(), state(), state()
            g0, g1, g2 = state(), state(), state()
            xF, gF = state(), state()
            s_sb = state()
            m_t = state()
            tt_ = state()
            isyn = state()
            inner = state()

            sT_cur = st_pool.tile([128, K_REC, BATCH], f32)
            nc.vector.memset(sT_cur[:], 0.0)

            stt = nc.vector.scalar_tensor_tensor
            stt_g = nc.vector.scalar_tensor_tensor

            for t in range(T):
                # FF matmul first: no dependence on the gathered spikes, so the
                # PE can chew on it while the previous step's AllGather lands.
                itT = it_pool.tile([128, K_FF, BATCH], f32)
                nc.sync.dma_start(itT[:], itT_in[:, :, t, :].rearrange("k p b -> p k b"))
                pff = pff_pool.tile([BATCH, COLS], f32)
                for k in range(K_FF):
                    nc.tensor.matmul(pff[:], itT[:, k, :], wf_sb[:, k, :],
                                     start=(k == 0), stop=(k == K_FF - 1))

                pinp = pin_pool.tile([BATCH, 2 * COLS], f32)
                for k in range(K_REC):
                    nc.tensor.matmul(pinp[:], sT_cur[:, k, :], w_sb[:, k, :],
                                     start=(k == 0), stop=(k == K_REC - 1))

                # refractory bookkeeping from previous step's state (no dep on
                # this step's matmul) — runs on Pool during the matmuls.
                nc.gpsimd.tensor_scalar(m_t[:], ref[:], 0.0, None, op0=op.is_gt)
                nc.gpsimd.tensor_scalar(ref[:], ref[:], -1.0, 0.0, op0=op.add, op1=op.max)

                # FF dual-exponential states
                stt(xF[:], xF[:], ARF, pff[:], op.mult, op.add)
                stt_g(gF[:], gF[:], ADF, xF[:], op.mult, op.add)

                # recurrent dual-exponential states
                stt(x0[:], x0[:], AR[0], pinp[:, 0:COLS], op.mult, op.add)
                stt(x1[:], x1[:], AR[1], pinp[:, 0:COLS], op.mult, op.add)
                stt(x2[:], x2[:], AR[2], pinp[:, COLS:2 * COLS], op.mult, op.add)
                stt_g(g0[:], g0[:], AD[0], x0[:], op.mult, op.add)
                stt_g(g1[:], g1[:], AD[1], x1[:], op.mult, op.add)
                stt(g2[:], g2[:], AD[2], x2[:], op.mult, op.add)

                # gtot = g0 + 0.5*g1 + g2 + gF   (gbar = [1, .5, 1], FF_GBAR=1)
                stt(tt_[:], g1[:], 0.5, g0[:], op.mult, op.add)
                stt_g(tt_[:], g2[:], 1.0, tt_[:], op.mult, op.add)
                stt(tt_[:], gF[:], 1.0, tt_[:], op.mult, op.add)
                # I_syn = -70*g2 - gtot*U   (gbarE = [0, 0, -70], FF_EREV=0)
                nc.vector.tensor_tensor(inner[:], tt_[:], U[:], op.mult)
                stt(isyn[:], g2[:], -70.0, inner[:], op.mult, op.subtract)
                # U += lc * (10*(-65-U) + I_syn) = lc * ((-10*U + I_syn) - 650)
                stt(inner[:], U[:], -10.0, isyn[:], op.mult, op.add)
                nc.vector.tensor_scalar(inner[:], inner[:], -650.0, None, op0=op.add)
                nc.vector.tensor_tensor(inner[:], inner[:], lc_t[:], op.mult)
                nc.vector.tensor_tensor(U[:], U[:], inner[:], op.add)
                # refractory clamp, spike, reset
                nc.vector.copy_predicated(U[:], m_t[:].bitcast(mybir.dt.int32), neg65[:])
                nc.vector.tensor_scalar(s_sb[:], U[:], -50.0, None, op0=op.is_ge)
                s_mask = s_sb[:].bitcast(mybir.dt.int32)
                nc.vector.copy_predicated(U[:], s_mask, neg65[:])
                nc.vector.copy_predicated(ref[:], s_mask, rs_t[:])

                if t < T - 1:
                    # transpose own spike slice to [neuron, batch] and gather
                    ptr = ptr_pool.tile([128, 2 * BATCH], f32)
                    nc.tensor.transpose(ptr[0:128, 0:BATCH], s_sb[:, 0:128], ident[:])
                    nc.tensor.transpose(ptr[0:64, BATCH:2 * BATCH],
                                        s_sb[:, 128:COLS], ident[:])
                    sp_st = st_pool.tile([128, 2 * BATCH], f32, tag="spst")
                    nc.scalar.copy(sp_st[:], ptr[:])
                    agi = agi_pool.tile([COLS, BATCH], f32)
                    nc.sync.dma_start(agi[0:128, :], sp_st[0:128, 0:BATCH])
                    nc.sync.dma_start(agi[128:COLS, :], sp_st[0:64, BATCH:2 * BATCH])
                    ago = ago_pool.tile([N_NEURONS, BATCH], f32)
                    nc.gpsimd.collective_compute(
                        "AllGather",
                        op.bypass,
                        replica_groups=[list(range(N_CORES))],
                        ins=[agi.opt()],
                        outs=[ago.opt()],
                    )
                    sT_cur = st_pool.tile([128, K_REC, BATCH], f32)
                    ago_v = ago.opt().rearrange("(k p) b -> p k b", p=128)
                    # 12 separate DMAs spread across HWDGE queues: each moves a
                    # contiguous 16KB k-tile, cutting the serial gather-return
                    # latency vs one strided transfer.
                    for k in range(K_REC):
                        nc.sync.dma_start(sT_cur[:, k, :], ago_v[:, k, :])

                nc.sync.dma_start(out_s[t], s_sb[:])
                nc.sync.dma_start(out_u[t], U[:])

    nc.compile()
    return nc


def _prep_inputs(input_spikes, weights, weights_FF, scaling_factors,
                 scaling_factors_FF, cell_type_indices, cell_type_indices_FF, T):
    ct = np.asarray(cell_type_indices).astype(np.int64)
    sf = np.asarray(scaling_factors, np.float32)[ct[:, None], ct[None, :]]
    W = np.asarray(weights, np.float32) * sf
    mask_e = (ct == 0).astype(np.float32)[:, None]
    W_e = W * mask_e
    W_i = W * (1.0 - mask_e)
    ctF = np.asarray(cell_type_indices_FF).astype(np.int64)
    sfF = np.asarray(scaling_factors_FF, np.float32)[ctF[:, None], ct[None, :]]
    WF = np.asarray(weights_FF, np.float32) * sfF

    tau_mem = CELL_TAU_MEM[ct]
    lc = (DT / (tau_mem * 10.0)).astype(np.float32)        # leak_coef per neuron
    rs = (CELL_TAUREF[ct] / DT).astype(np.float32)          # refractory steps

    isp = np.ascontiguousarray(np.asarray(input_spikes, np.float32)[:, :T, :])
    # itT[k, p, t, b] = input_spikes[b, t, 128k+p]
    itT = np.ascontiguousarray(
        isp.transpose(2, 1, 0).reshape(K_FF, 128, T, BATCH))

    ident = np.eye(BATCH, dtype=np.float32)

    in_maps = []
    for c in range(N_CORES):
        cols = slice(c * COLS, (c + 1) * COLS)
        wcat = np.concatenate([W_e[:, cols], W_i[:, cols]], axis=1)  # (1536, 384)
        w_in = np.ascontiguousarray(wcat.reshape(K_REC, 128, 2 * COLS))
        wf_c = np.ascontiguousarray(WF[:, cols].reshape(K_FF, 128, COLS))
        lc_c = np.broadcast_to(lc[cols], (BATCH, COLS)).copy()
        rs_c = np.broadcast_to(rs[cols], (BATCH, COLS)).copy()
        in_maps.append({
            "w_in": w_in,
            "wf_in": wf_c,
            "itT_in": itT,
            "lc_in": lc_c,
            "rs_in": rs_c,
            "id_in": ident,
        })
    return in_maps


_NC_CACHE = {}


def run(inputs: dict, T: int = T_STEPS, trace: bool = False):
    from concourse.bass_utils import run_bass_kernel_spmd

    if T not in _NC_CACHE:
        _NC_CACHE[T] = _build(T)
    nc = _NC_CACHE[T]
    in_maps = _prep_inputs(T=T, **inputs)
    res = run_bass_kernel_spmd(
        nc, in_maps, core_ids=list(range(N_CORES)), trace=trace,
    )
    spk = np.concatenate([r["out_s"] for r in res.results], axis=2)
    volts = np.concatenate([r["out_u"] for r in res.results], axis=2)
    spk = np.ascontiguousarray(spk.transpose(1, 0, 2))
    volts = np.ascontiguousarray(volts.transpose(1, 0, 2))
    return (spk, volts), res


def kernel(**inputs):
    (spk, volts), _ = run(inputs, T=T_STEPS, trace=False)
    return spk, volts
